# revision 1
# baseline (speedup 1.0000x reference)
"""MLA (CustomLlamaMLAForInfer) Trainium2 Bass kernel.

Sharding: tensor-parallel over heads across 8 NeuronCores. Core c owns
kv-head c and q-heads [4c, 4c+4). Every core sees the full token stream
(B*S = 4096 tokens); o_proj is computed against the core's 512
head-dims, producing a partial [4096, 4096] output that the host sums
across the 8 cores.

Device program phases (single SPMD program, per-core weights differ):
  1a. qT = Wq_shard @ hidden.T   (rope + 1/sqrt(d) folded in at evict)
  1b. c_kvT = Wdk @ hidden.T ; krT = Wkr_shard @ hidden.T (rope at evict)
  2.  k_c / v from c_kvT via Wupk/Wupv shards; assemble kT_full, v_tok
  3.  causal attention per (batch, q-head): scores_T = kT.T@qT blocks,
      exp (no max-sub needed: |scores| < ~6), mask diag blocks,
      out_T[d,q] += v_tok.T @ p_T, sums via ones-matmul, normalize
  4.  partial o_proj: out[tok, hid] += attn_T.T @ WoT_shard

All matmuls run as float32r (fp22 mantissa, 1 PE pass).
"""

import numpy as np

HIDDEN = 4096
N_HEADS = 32
KV_HEADS = 8
HEAD_DIM = 128
LOW_RANK = 64
TOP_K_ROPE = 32
ROPE_THETA = 10000.0
B, S = 2, 2048
NCORES = 8
HPC = N_HEADS // NCORES          # q heads per core = 4
QR = HPC * HEAD_DIM              # q rows per core = 512
CD = LOW_RANK * KV_HEADS         # latent dim = 512
KRR = 2 * TOP_K_ROPE             # rope rows per kv head = 64


def _rope_tables(seq_len):
    inv = 1.0 / (ROPE_THETA ** (np.arange(0, HEAD_DIM, 2, dtype=np.float32) / HEAD_DIM))
    pos = np.arange(seq_len, dtype=np.float32)
    fr = np.outer(pos, inv)
    emb = np.concatenate([fr, fr], axis=-1)          # [S, 128]
    return (np.cos(emb).T.astype(np.float32),        # [128, S]
            np.sin(emb).T.astype(np.float32))


def build_program(Bv=B, Sv=S, TB=512, QB=512, trace_sim=False):
    """Build the SPMD Bass program. TB = proj token-block, QB = attention
    q-block (both <= 512, the fp32 moving-operand limit)."""
    from concourse import bacc, tile, mybir
    import concourse.bass as bass

    f32 = mybir.dt.float32
    F32R = mybir.dt.float32r
    MS = bass.MemorySpace
    EXP = mybir.ActivationFunctionType.Exp

    NT = Bv * Sv                 # total tokens
    HT = HIDDEN // 128           # hidden tiles = 32
    NTB = NT // TB               # proj token blocks
    NQB = Sv // QB               # q blocks per batch
    NJ = QB // 128               # diagonal mask variants
    NKT_B = Sv // 128            # k tiles per batch
    QT = QR // 128               # q-head tiles per core = 4
    LT = CD // 128               # latent tiles = 4

    nc = bacc.Bacc("TRN2", target_bir_lowering=False, debug=False,
                   num_devices=NCORES)

    def din(name, shape):
        return nc.dram_tensor(name, shape, f32, kind="ExternalInput").ap()

    hidT = din("hidT", [HIDDEN, NT])
    wq = din("wq_t", [HIDDEN, QR])
    wkd = din("wkd_t", [HIDDEN, CD + KRR])
    wupk = din("wupk_t", [CD, KRR])
    wupv = din("wupv_t", [CD, HEAD_DIM])
    wo = din("wo_t", [QR, HIDDEN])
    qcos = din("qcos", [128, NT])
    qsin = din("qsin", [128, NT])
    kcos = din("kcos", [KRR, NT])
    ksin = din("ksin", [KRR, NT])
    masks = din("masks", [128, NJ, QB])
    onesd = din("ones", [128, 1])
    outp = nc.dram_tensor("out_part", [NT, HIDDEN], f32, kind="ExternalOutput").ap()
    qT_s = nc.dram_tensor("qT_s", [QT, 128, NT], f32).ap()
    ckv_s = nc.dram_tensor("ckv_s", [LT, 128, NT], f32).ap()

    with tile.TileContext(nc, trace_sim=trace_sim) as tc:
        with tc.tile_pool(name="persist", bufs=1) as pers:
            kT_full = pers.tile([128, NT], F32R, tag="kT")
            v_tok = pers.tile([128, NT // 128, HEAD_DIM], F32R, tag="vtok")

            # ---------------- phase 1: projections of hidden ----------------
            with tc.tile_pool(name="cos", bufs=1) as cp:
                qcos_sb = cp.tile([128, NT], f32, tag="qc")
                qsin_sb = cp.tile([128, NT], f32, tag="qs")
                kcos_sb = cp.tile([KRR, NT], f32, tag="kc")
                ksin_sb = cp.tile([KRR, NT], f32, tag="ks")
                nc.sync.dma_start(qcos_sb[:], qcos)
                nc.sync.dma_start(qsin_sb[:], qsin)
                nc.sync.dma_start(kcos_sb[:], kcos)
                nc.sync.dma_start(ksin_sb[:], ksin)

                # ---- pass A: q projection (+rope, +1/sqrt(d) via tables) ----
                with tc.tile_pool(name="wqp", bufs=1) as wqp, \
                     tc.tile_pool(name="hidA", bufs=8) as hpA, \
                     tc.tile_pool(name="stA", bufs=2) as stA, \
                     tc.tile_pool(name="psA", bufs=8, space=MS.PSUM) as ppA:
                    wq_sb = wqp.tile([128, HT, QR], F32R)
                    nc.sync.dma_start(wq_sb[:], wq.rearrange("(t p) w -> p t w", p=128).bitcast(F32R))
                    for blk in range(NTB):
                        c0, c1 = blk * TB, (blk + 1) * TB
                        qps = [ppA.tile([128, TB], f32, tag="qps", name=f"qps{_m}") for _m in range(QT)]
                        for t in range(HT):
                            ht = hpA.tile([128, TB], F32R, tag="hid")
                            nc.sync.dma_start(ht[:], hidT[t * 128:(t + 1) * 128, c0:c1].bitcast(F32R))
                            for m in range(QT):
                                nc.tensor.matmul(
                                    qps[m][:],
                                    wq_sb[:, t, m * 128:(m + 1) * 128],
                                    ht[:],
                                    start=(t == 0), stop=(t == HT - 1))
                        for m in range(QT):
                            raw = stA.tile([128, TB], f32, tag="raw")
                            nc.scalar.copy(raw[:], qps[m][:])
                            rot = stA.tile([128, TB], f32, tag="rot")
                            nc.sync.dma_start(rot[0:64, :], raw[64:128, :])
                            nc.sync.dma_start(rot[64:128, :], raw[0:64, :])
                            qsb = stA.tile([128, TB], f32, tag="qsb")
                            nc.vector.tensor_mul(qsb[:], raw[:], qcos_sb[:, c0:c1])
                            nc.vector.tensor_mul(rot[:], rot[:], qsin_sb[:, c0:c1])
                            nc.vector.tensor_add(qsb[:], qsb[:], rot[:])
                            nc.sync.dma_start(qT_s[m, :, c0:c1], qsb[:])

                # ---- pass B: c_kv (latent) + k_rope projections ----
                with tc.tile_pool(name="wkdp", bufs=1) as wkdp, \
                     tc.tile_pool(name="hidB", bufs=8) as hpB, \
                     tc.tile_pool(name="stB", bufs=2) as stB, \
                     tc.tile_pool(name="psB", bufs=6, space=MS.PSUM) as ppB, \
                     tc.tile_pool(name="psBk", bufs=2, space=MS.PSUM) as ppBk:
                    wkd_sb = wkdp.tile([128, HT, CD + KRR], F32R)
                    nc.sync.dma_start(wkd_sb[:], wkd.rearrange("(t p) w -> p t w", p=128).bitcast(F32R))
                    for blk in range(NTB):
                        c0, c1 = blk * TB, (blk + 1) * TB
                        dps = [ppB.tile([128, TB], f32, tag="dps", name=f"dps{_m}") for _m in range(LT)]
                        krp = ppBk.tile([KRR, TB], f32, tag="krp")
                        for t in range(HT):
                            ht = hpB.tile([128, TB], F32R, tag="hid")
                            nc.sync.dma_start(ht[:], hidT[t * 128:(t + 1) * 128, c0:c1].bitcast(F32R))
                            for m in range(LT):
                                nc.tensor.matmul(
                                    dps[m][:],
                                    wkd_sb[:, t, m * 128:(m + 1) * 128],
                                    ht[:],
                                    start=(t == 0), stop=(t == HT - 1))
                            nc.tensor.matmul(
                                krp[:],
                                wkd_sb[:, t, CD:CD + KRR],
                                ht[:],
                                start=(t == 0), stop=(t == HT - 1))
                        for m in range(LT):
                            csb = stB.tile([128, TB], f32, tag="csb")
                            nc.scalar.copy(csb[:], dps[m][:])
                            nc.sync.dma_start(ckv_s[m, :, c0:c1], csb[:])
                        # rope the 64 k-rope rows, scatter into kT_full
                        rawk = stB.tile([KRR, TB], f32, tag="rawk")
                        nc.scalar.copy(rawk[:], krp[:])
                        rotk = stB.tile([KRR, TB], f32, tag="rotk")
                        nc.sync.dma_start(rotk[0:32, :], rawk[32:64, :])
                        nc.sync.dma_start(rotk[32:64, :], rawk[0:32, :])
                        ksb = stB.tile([KRR, TB], f32, tag="ksb")
                        nc.vector.tensor_mul(ksb[:], rawk[:], kcos_sb[:, c0:c1])
                        nc.vector.tensor_mul(rotk[:], rotk[:], ksin_sb[:, c0:c1])
                        nc.vector.tensor_add(ksb[:], ksb[:], rotk[:])
                        nc.sync.dma_start(kT_full[0:32, c0:c1], ksb[0:32, :].bitcast(F32R))
                        nc.sync.dma_start(kT_full[64:96, c0:c1], ksb[32:64, :].bitcast(F32R))

            # ---------------- phase 2: k_c and v from the latent ----------------
            with tc.tile_pool(name="wup", bufs=1) as wup, \
                 tc.tile_pool(name="ckvb", bufs=2) as ckvb, \
                 tc.tile_pool(name="st2", bufs=2) as st2, \
                 tc.tile_pool(name="psK", bufs=2, space=MS.PSUM) as psK, \
                 tc.tile_pool(name="psV", bufs=4, space=MS.PSUM) as psV:
                wupk_sb = wup.tile([128, LT, KRR], F32R, tag="upk")
                wupv_sb = wup.tile([128, LT, HEAD_DIM], F32R, tag="upv")
                nc.sync.dma_start(wupk_sb[:], wupk.rearrange("(t p) w -> p t w", p=128).bitcast(F32R))
                nc.sync.dma_start(wupv_sb[:], wupv.rearrange("(t p) w -> p t w", p=128).bitcast(F32R))
                for blk in range(NTB):
                    c0, c1 = blk * TB, (blk + 1) * TB
                    cb = ckvb.tile([128, LT, TB], F32R, tag="cb")
                    nc.sync.dma_start(cb[:], ckv_s[:, :, c0:c1].rearrange("t p w -> p t w").bitcast(F32R))
                    kcp = psK.tile([KRR, TB], f32, tag="kcp")
                    for lt in range(LT):
                        nc.tensor.matmul(kcp[:],
                                         wupk_sb[:, lt, :],
                                         cb[:, lt, :],
                                         start=(lt == 0), stop=(lt == LT - 1))
                    kcs = st2.tile([KRR, TB], f32, tag="kcs")
                    nc.scalar.copy(kcs[:], kcp[:])
                    nc.sync.dma_start(kT_full[32:64, c0:c1], kcs[0:32, :].bitcast(F32R))
                    nc.sync.dma_start(kT_full[96:128, c0:c1], kcs[32:64, :].bitcast(F32R))
                    for tt in range(TB // 128):
                        vp = psV.tile([128, HEAD_DIM], f32, tag="vp")
                        for lt in range(LT):
                            nc.tensor.matmul(
                                vp[:],
                                cb[:, lt, tt * 128:(tt + 1) * 128],
                                wupv_sb[:, lt, :],
                                start=(lt == 0), stop=(lt == LT - 1))
                        nc.scalar.copy(v_tok[:, blk * (TB // 128) + tt, :], vp[:])

            # ---------------- phases 3+4 ----------------
            with tc.tile_pool(name="attn", bufs=1) as ap_:
                attn_sb = ap_.tile([128, QT, NT], F32R)

                with tc.tile_pool(name="qh", bufs=2) as qhp, \
                     tc.tile_pool(name="cst3", bufs=1) as cst3, \
                     tc.tile_pool(name="pt", bufs=3) as ptp, \
                     tc.tile_pool(name="sm", bufs=2) as smp, \
                     tc.tile_pool(name="psS", bufs=3, space=MS.PSUM) as psS, \
                     tc.tile_pool(name="psO", bufs=2, space=MS.PSUM) as psO, \
                     tc.tile_pool(name="psU", bufs=2, space=MS.PSUM) as psU:
                    masks_sb = cst3.tile([128, NJ, QB], F32R, tag="masks")
                    nc.sync.dma_start(masks_sb[:], masks.bitcast(F32R))
                    ones_sb = cst3.tile([128, 1], F32R, tag="ones")
                    nc.sync.dma_start(ones_sb[:], onesd.bitcast(F32R))
                    for h in range(QT):
                        qh_sb = qhp.tile([128, NT], F32R, tag="qh")
                        nc.sync.dma_start(qh_sb[:], qT_s[h].bitcast(F32R))
                        for b in range(Bv):
                            off = b * Sv
                            for qb in range(NQB):
                                ops = psO.tile([128, QB], f32, tag="ops")
                                sps = psU.tile([1, QB], f32, tag="sps")
                                nkt = (qb + 1) * NJ
                                for kt in range(nkt):
                                    scp = psS.tile([128, QB], f32, tag="scp")
                                    nc.tensor.matmul(
                                        scp[:],
                                        kT_full[:, off + kt * 128: off + (kt + 1) * 128],
                                        qh_sb[:, off + qb * QB: off + (qb + 1) * QB],
                                        start=True, stop=True)
                                    ptile = ptp.tile([128, QB], F32R, tag="pt")
                                    nc.scalar.activation(ptile[:], scp[:], EXP)
                                    j = kt - qb * NJ
                                    if j >= 0:
                                        nc.vector.tensor_mul(ptile[:], ptile[:], masks_sb[:, j, :])
                                    nc.tensor.matmul(
                                        ops[:],
                                        v_tok[:, b * NKT_B + kt, :],
                                        ptile[:],
                                        start=(kt == 0), stop=(kt == nkt - 1))
                                    nc.tensor.matmul(
                                        sps[:],
                                        ones_sb[:],
                                        ptile[:],
                                        start=(kt == 0), stop=(kt == nkt - 1))
                                rec = smp.tile([1, QB], f32, tag="rec")
                                nc.vector.reciprocal(rec[:], sps[:])
                                rb = smp.tile([128, QB], f32, tag="rb")
                                nc.gpsimd.partition_broadcast(rb[:], rec[:])
                                nc.vector.tensor_mul(
                                    attn_sb[:, h, off + qb * QB: off + (qb + 1) * QB],
                                    ops[:], rb[:])

                # ---- phase 4: partial o_proj ----
                with tc.tile_pool(name="wop", bufs=1) as wop, \
                     tc.tile_pool(name="st4", bufs=4) as st4, \
                     tc.tile_pool(name="ps4", bufs=6, space=MS.PSUM) as ps4:
                    wo_sb = wop.tile([128, QT, HIDDEN], F32R)
                    nc.sync.dma_start(wo_sb[:], wo.rearrange("(t p) w -> p t w", p=128).bitcast(F32R))
                    for T in range(NT // 128):
                        for n in range(HIDDEN // 512):
                            ps = ps4.tile([128, 512], f32, tag="ps")
                            for h2 in range(QT):
                                nc.tensor.matmul(
                                    ps[:],
                                    attn_sb[:, h2, T * 128:(T + 1) * 128],
                                    wo_sb[:, h2, n * 512:(n + 1) * 512],
                                    start=(h2 == 0), stop=(h2 == QT - 1))
                            osb = st4.tile([128, 512], f32, tag="osb")
                            nc.vector.tensor_copy(osb[:], ps[:])
                            nc.sync.dma_start(outp[T * 128:(T + 1) * 128, n * 512:(n + 1) * 512], osb[:])

    nc.compile()
    return nc


def make_in_maps(hidden_states, Wq, Wkr, Wdk, Wupk, Wupv, Wo, Bv=B, Sv=S, QB=512):
    """Host-side sharding + layout prep. Returns per-core input dicts."""
    NT = Bv * Sv
    NJ = QB // 128
    scale = 1.0 / np.sqrt(np.float32(HEAD_DIM))

    hidT = np.ascontiguousarray(
        hidden_states.reshape(NT, HIDDEN).T.astype(np.float32))

    cos_t, sin_t = _rope_tables(Sv)                    # [128, S]
    cos_t = np.tile(cos_t, (1, Bv))                    # [128, NT]
    sin_t = np.tile(sin_t, (1, Bv))
    qcos = np.ascontiguousarray(cos_t * scale)
    qsin = np.ascontiguousarray(
        np.concatenate([-sin_t[0:64], sin_t[64:128]], axis=0) * scale)
    kcos = np.ascontiguousarray(
        np.concatenate([cos_t[0:32], cos_t[64:96]], axis=0))
    ksin = np.ascontiguousarray(
        np.concatenate([-sin_t[0:32], sin_t[64:96]], axis=0))

    k_idx = np.arange(128)[:, None]
    q_idx = np.arange(QB)[None, :]
    masks = np.stack(
        [(q_idx >= j * 128 + k_idx).astype(np.float32) for j in range(NJ)],
        axis=1)                                        # [128, NJ, QB]
    masks = np.ascontiguousarray(masks)

    in_maps = []
    for c in range(NCORES):
        wq_t = np.ascontiguousarray(Wq[QR * c:QR * (c + 1)].T.astype(np.float32))
        wkd_t = np.ascontiguousarray(
            np.concatenate([Wdk, Wkr[KRR * c:KRR * (c + 1)]], axis=0).T.astype(np.float32))
        wupk_t = np.ascontiguousarray(Wupk[KRR * c:KRR * (c + 1)].T.astype(np.float32))
        wupv_t = np.ascontiguousarray(
            Wupv[HEAD_DIM * c:HEAD_DIM * (c + 1)].T.astype(np.float32))
        wo_t = np.ascontiguousarray(Wo[:, QR * c:QR * (c + 1)].T.astype(np.float32))
        in_maps.append({
            "hidT": hidT, "wq_t": wq_t, "wkd_t": wkd_t,
            "wupk_t": wupk_t, "wupv_t": wupv_t, "wo_t": wo_t,
            "qcos": qcos, "qsin": qsin, "kcos": kcos, "ksin": ksin,
            "masks": masks, "ones": np.ones((128, 1), np.float32),
        })
    return in_maps


_NC_CACHE = {}


def _get_program(key=(B, S, 512, 512)):
    if key not in _NC_CACHE:
        _NC_CACHE[key] = build_program(*key)
    return _NC_CACHE[key]


def kernel(hidden_states, Wq, Wkr, Wdk, Wupk, Wupv, Wo):
    from concourse.bass_utils import run_bass_kernel_spmd

    hidden_states = np.asarray(hidden_states)
    in_maps = make_in_maps(hidden_states, np.asarray(Wq), np.asarray(Wkr),
                           np.asarray(Wdk), np.asarray(Wupk), np.asarray(Wupv),
                           np.asarray(Wo))
    nc = _get_program()
    res = run_bass_kernel_spmd(nc, in_maps, list(range(NCORES)))
    out = res.results[0]["out_part"].astype(np.float32)
    for i in range(1, NCORES):
        out = out + res.results[i]["out_part"]
    return out.reshape(B, S, HIDDEN).astype(np.float32)



# revision 5
# speedup vs baseline: 1.2050x; 1.2050x over previous
"""MLA (CustomLlamaMLAForInfer) Trainium2 Bass kernel, v2.

Sharding: tensor-parallel over heads across 8 NeuronCores. Core c owns
kv-head c and q-heads [4c, 4c+4). Every core sees the full token stream
(B*S = 4096 tokens); o_proj is computed against the core's 512
head-dims, producing a partial [4096, 4096] bf16 output that the host
sums across the 8 cores.

v2 changes vs baseline:
  - Host fuses Wupk/Wupv through Wdk (k_c = hid @ (Wupk_c Wdk).T etc.),
    removing the replicated 512-dim latent projection and its DRAM
    round trip entirely.
  - Single phase-1 pass over hidT: one 6-bank PSUM group per token
    block produces q (4 tiles), interleaved k_rope/k_nope (1 tile,
    weight columns pre-permuted so no cross-partition moves at evict),
    vT (1 tile, PE-transposed to [tok, d]).
  - bf16 operands on the PE except p/v (f32r), halving DMA traffic.
  - qT stays resident in SBUF (no DRAM round trip).
  - Attention: scores for 2 k-tiles accumulate into one 2-bank PSUM
    tile, one wide exp (N=1024) per group; softmax denominators via
    ones-matmul; reciprocal_approx_fast instead of iterative reciprocal.
  - o_proj interleaved per (b, qb) block right after its 4 heads
    finish, sharing PSUM banks with the scores pool; qb descending so
    the wo prefetch hides under the deepest attention block.
"""

import numpy as np

HIDDEN = 4096
N_HEADS = 32
KV_HEADS = 8
HEAD_DIM = 128
LOW_RANK = 64
TOP_K_ROPE = 32
ROPE_THETA = 10000.0
B, S = 2, 2048
NCORES = 8
HPC = N_HEADS // NCORES          # q heads per core = 4
QR = HPC * HEAD_DIM              # q rows per core = 512
CD = LOW_RANK * KV_HEADS         # latent dim = 512
KRR = 2 * TOP_K_ROPE             # rope rows per kv head = 64
WKV = 256                        # fused kv out rows: kr 64 + kc 64 + v 128
WC = QR + WKV                    # combined projection out rows = 768


def _rope_tables(seq_len):
    inv = 1.0 / (ROPE_THETA ** (np.arange(0, HEAD_DIM, 2, dtype=np.float32) / HEAD_DIM))
    pos = np.arange(seq_len, dtype=np.float32)
    fr = np.outer(pos, inv)
    emb = np.concatenate([fr, fr], axis=-1)          # [S, 128]
    return (np.cos(emb).T.astype(np.float32),        # [128, S]
            np.sin(emb).T.astype(np.float32))


def build_program(Bv=B, Sv=S, TB=512, QB=512, trace_sim=False):
    from concourse import bacc, tile, mybir
    import concourse.bass as bass

    f32 = mybir.dt.float32
    F32R = mybir.dt.float32r
    BF16 = mybir.dt.bfloat16
    MS = bass.MemorySpace
    EXP = mybir.ActivationFunctionType.Exp

    NT = Bv * Sv                 # total tokens = 4096
    HT = HIDDEN // 128           # hidden tiles = 32
    NTB = NT // TB               # proj token blocks = 8
    NQB = Sv // QB               # q blocks per batch = 4
    NJ = QB // 128               # diagonal mask variants = 4
    NKT_B = Sv // 128            # k tiles per batch = 16
    QT = QR // 128               # q-head tiles per core = 4

    nc = bacc.Bacc("TRN2", target_bir_lowering=False, debug=False,
                   num_devices=NCORES)

    def din(name, shape, dt=BF16):
        return nc.dram_tensor(name, shape, dt, kind="ExternalInput").ap()

    hidT = din("hidT", [HIDDEN, NT])
    wcomb = din("wcomb", [HIDDEN, WC])
    wo = din("wo_t", [QR, HIDDEN])
    ropes = din("ropes", [128, 4, NT])   # 0=qcos 1=qsin 2=kcos 3=ksin
    masks = din("masks", [128, NJ, QB], f32)
    onesd = din("ones", [128, 1], f32)
    identd = din("ident", [128, 128], f32)
    outp = nc.dram_tensor("out_part", [NT, HIDDEN], BF16, kind="ExternalOutput").ap()

    with tile.TileContext(nc, trace_sim=trace_sim) as tc:
        with tc.tile_pool(name="persist", bufs=1) as pers:
            kT = pers.tile([128, NT], BF16, tag="kT")
            qT = pers.tile([128, QT, NT], BF16, tag="qT")
            v_tok = pers.tile([128, NT // 128, HEAD_DIM], F32R, tag="vtok")

            # ---------------- phase 1: fused projections of hidden ----------
            with tc.tile_pool(name="p1c", bufs=1) as cp, \
                 tc.tile_pool(name="hid", bufs=3) as hp, \
                 tc.tile_pool(name="rps", bufs=2) as rpp, \
                 tc.tile_pool(name="st1", bufs=2) as st, \
                 tc.tile_pool(name="ps1", bufs=6, space=MS.PSUM) as pp, \
                 tc.tile_pool(name="psT", bufs=2, space=MS.PSUM) as pvt:
                ident_sb = cp.tile([128, 128], F32R, tag="id")
                nc.sync.dma_start(ident_sb[:], identd.bitcast(F32R))
                wc_sb = cp.tile([128, HT, WC], BF16, tag="wc")
                nc.sync.dma_start(wc_sb[:], wcomb.rearrange("(t p) w -> p t w", p=128))

                for blk in range(NTB):
                    c0, c1 = blk * TB, (blk + 1) * TB
                    rp = rpp.tile([128, 4, TB], BF16, tag="rp")
                    nc.sync.dma_start(rp[:], ropes[:, :, c0:c1])
                    hts = []
                    for half in range(2):
                        ht = hp.tile([128, HT // 2, TB], BF16, tag="hid")
                        nc.sync.dma_start(
                            ht[:],
                            hidT[half * 2048:(half + 1) * 2048, c0:c1]
                            .rearrange("(t p) w -> p t w", p=128))
                        hts.append(ht)
                    ps = [pp.tile([128, TB], f32, tag="ps1", name=f"ps{_m}")
                          for _m in range(6)]
                    for t in range(HT):
                        htt = hts[t // 16][:, t % 16, :]
                        for m in range(6):
                            nc.tensor.matmul(
                                ps[m][:],
                                wc_sb[:, t, m * 128:(m + 1) * 128],
                                htt,
                                start=(t == 0), stop=(t == HT - 1))
                    # ---- evict q tiles (rope via sign-folded tables) ----
                    for m in range(QT):
                        qraw = st.tile([128, TB], BF16, tag="qraw")
                        nc.scalar.copy(qraw[:], ps[m][:])
                        rot = st.tile([128, TB], BF16, tag="rot")
                        nc.sync.dma_start(rot[0:64, :], qraw[64:128, :])
                        nc.sync.dma_start(rot[64:128, :], qraw[0:64, :])
                        t1 = st.tile([128, TB], BF16, tag="t1")
                        nc.vector.tensor_mul(t1[:], qraw[:], rp[:, 0, :])
                        nc.vector.tensor_mul(rot[:], rot[:], rp[:, 1, :])
                        nc.vector.tensor_add(qT[:, m, c0:c1], t1[:], rot[:])
                    # ---- evict kv tile: rows 0:32 krA, 32:64 kcA,
                    #      64:96 krB, 96:128 kcB (host pre-permuted) ----
                    kk = st.tile([128, TB], BF16, tag="kk")
                    nc.scalar.copy(kk[:], ps[4][:])
                    rt = st.tile([128, TB], BF16, tag="rt")
                    nc.sync.dma_start(rt[0:32, :], kk[64:96, :])
                    nc.sync.dma_start(rt[64:96, :], kk[0:32, :])
                    t2 = st.tile([128, TB], BF16, tag="t2")
                    for r0 in (0, 64):
                        nc.vector.tensor_mul(
                            t2[r0:r0 + 32, :], kk[r0:r0 + 32, :],
                            rp[r0:r0 + 32, 2, :])
                        nc.vector.tensor_mul(
                            rt[r0:r0 + 32, :], rt[r0:r0 + 32, :],
                            rp[r0:r0 + 32, 3, :])
                        nc.vector.tensor_add(
                            kT[r0:r0 + 32, c0:c1], t2[r0:r0 + 32, :],
                            rt[r0:r0 + 32, :])
                    nc.scalar.copy(kT[32:64, c0:c1], kk[32:64, :])
                    nc.scalar.copy(kT[96:128, c0:c1], kk[96:128, :])
                    # ---- evict v tile: vT [d, tok] -> transpose to [tok, d] --
                    vst = st.tile([128, TB], F32R, tag="vst")
                    nc.scalar.copy(vst[:], ps[5][:])
                    for tt in range(TB // 128):
                        vtp = pvt.tile([128, 128], F32R, tag="vtp")
                        nc.tensor.transpose(
                            vtp[:], vst[:, tt * 128:(tt + 1) * 128], ident_sb[:])
                        nc.vector.tensor_copy(
                            v_tok[:, blk * (TB // 128) + tt, :], vtp[:])

            # ---------------- phases 3+4 interleaved ----------------
            with tc.tile_pool(name="wop", bufs=1) as wop, \
                 tc.tile_pool(name="attn", bufs=1) as ap_, \
                 tc.tile_pool(name="c3", bufs=1) as cst3, \
                 tc.tile_pool(name="pt", bufs=3) as ptp, \
                 tc.tile_pool(name="sm", bufs=2) as smp, \
                 tc.tile_pool(name="st4", bufs=2) as st4, \
                 tc.tile_pool(name="psS", bufs=2, space=MS.PSUM) as psS, \
                 tc.tile_pool(name="psO", bufs=2, space=MS.PSUM) as psO, \
                 tc.tile_pool(name="psU", bufs=2, space=MS.PSUM) as psU:
                wo_sb = wop.tile([128, QT, HIDDEN], BF16, tag="wo")
                nc.sync.dma_start(wo_sb[:], wo.rearrange("(t p) w -> p t w", p=128))
                attn_sb = ap_.tile([128, QT, NT], BF16, tag="attn")
                masks_sb = cst3.tile([128, NJ, QB], F32R, tag="masks")
                nc.sync.dma_start(masks_sb[:], masks.bitcast(F32R))
                ones_sb = cst3.tile([128, 1], F32R, tag="ones")
                nc.sync.dma_start(ones_sb[:], onesd.bitcast(F32R))

                for b in range(Bv):
                    off = b * Sv
                    for qb in range(NQB - 1, -1, -1):
                        q0 = off + qb * QB
                        ngrp = NJ * (qb + 1) // 2   # groups of 2 k-tiles
                        for h in range(QT):
                            ops = psO.tile([128, QB], f32, tag="ops")
                            sps = psU.tile([1, QB], f32, tag="sps")
                            for g in range(ngrp):
                                scp = psS.tile([128, 2, QB], f32, tag="scp")
                                for j2 in range(2):
                                    kt = 2 * g + j2
                                    nc.tensor.matmul(
                                        scp[:, j2, :],
                                        kT[:, off + kt * 128: off + (kt + 1) * 128],
                                        qT[:, h, q0:q0 + QB],
                                        start=True, stop=True)
                                ptile = ptp.tile([128, 2, QB], F32R, tag="pt")
                                nc.scalar.activation(ptile[:], scp[:], EXP)
                                dg = g - NJ * qb // 2
                                if dg >= 0:   # diagonal groups: causal mask
                                    nc.vector.tensor_mul(
                                        ptile[:], ptile[:],
                                        masks_sb[:, 2 * dg:2 * dg + 2, :])
                                for j2 in range(2):
                                    kt = 2 * g + j2
                                    first = (g == 0 and j2 == 0)
                                    last = (g == ngrp - 1 and j2 == 1)
                                    nc.tensor.matmul(
                                        ops[:],
                                        v_tok[:, b * NKT_B + kt, :],
                                        ptile[:, j2, :],
                                        start=first, stop=last)
                                    nc.tensor.matmul(
                                        sps[:],
                                        ones_sb[:],
                                        ptile[:, j2, :],
                                        start=first, stop=last)
                            rec = smp.tile([1, QB], f32, tag="rec")
                            nc.vector.reciprocal_approx_fast(rec[:], sps[:])
                            rb = smp.tile([128, QB], f32, tag="rb")
                            nc.gpsimd.partition_broadcast(rb[:], rec[:])
                            nc.vector.tensor_mul(
                                attn_sb[:, h, q0:q0 + QB], ops[:], rb[:])
                        # ---- o_proj for this token block (all 4 heads done) --
                        for Tt in range(QB // 128):
                            T0 = q0 + Tt * 128
                            orow = st4.tile([128, HIDDEN], BF16, tag="orow")
                            for npair in range(HIDDEN // 1024):
                                po = psS.tile([128, 2, QB], f32, tag="scp")
                                for half in range(2):
                                    nn = 2 * npair + half
                                    for h2 in range(QT):
                                        nc.tensor.matmul(
                                            po[:, half, :],
                                            attn_sb[:, h2, T0:T0 + 128],
                                            wo_sb[:, h2, nn * 512:(nn + 1) * 512],
                                            start=(h2 == 0), stop=(h2 == QT - 1))
                                # alternate eviction engine to balance load
                                if npair % 2 == 0:
                                    nc.vector.tensor_copy(
                                        orow[:, npair * 1024:(npair + 1) * 1024],
                                        po[:])
                                else:
                                    nc.scalar.copy(
                                        orow[:, npair * 1024:(npair + 1) * 1024],
                                        po[:])
                            nc.sync.dma_start(outp[T0:T0 + 128, :], orow[:])

    nc.compile()
    return nc


def make_in_maps(hidden_states, Wq, Wkr, Wdk, Wupk, Wupv, Wo, Bv=B, Sv=S, QB=512):
    """Host-side sharding + layout prep. Returns per-core input dicts."""
    import ml_dtypes
    bf16 = ml_dtypes.bfloat16

    NT = Bv * Sv
    NJ = QB // 128
    scale = 1.0 / np.sqrt(np.float32(HEAD_DIM))

    hidden_states = np.asarray(hidden_states, dtype=np.float32)
    Wq = np.asarray(Wq, dtype=np.float32)
    Wkr = np.asarray(Wkr, dtype=np.float32)
    Wdk = np.asarray(Wdk, dtype=np.float32)
    Wupk = np.asarray(Wupk, dtype=np.float32)
    Wupv = np.asarray(Wupv, dtype=np.float32)
    Wo = np.asarray(Wo, dtype=np.float32)

    hidT = np.ascontiguousarray(
        hidden_states.reshape(NT, HIDDEN).T).astype(bf16)

    cos_t, sin_t = _rope_tables(Sv)                    # [128, S]
    cos_t = np.tile(cos_t, (1, Bv))                    # [128, NT]
    sin_t = np.tile(sin_t, (1, Bv))
    qcos = cos_t * scale
    qsin = np.concatenate([-sin_t[0:64], sin_t[64:128]], axis=0) * scale
    # k rope tables in the permuted kv-row layout:
    #   rows 0:32  = rotated rope dims 0:32   (krA):  cos[0:32],  -sin[0:32]
    #   rows 64:96 = rotated rope dims 64:96  (krB):  cos[64:96], +sin[64:96]
    kcos = np.zeros((128, NT), np.float32)
    ksin = np.zeros((128, NT), np.float32)
    kcos[0:32] = cos_t[0:32]
    kcos[64:96] = cos_t[64:96]
    ksin[0:32] = -sin_t[0:32]
    ksin[64:96] = sin_t[64:96]
    ropes = np.ascontiguousarray(
        np.stack([qcos, qsin, kcos, ksin], axis=1)).astype(bf16)  # [128,4,NT]

    k_idx = np.arange(128)[:, None]
    q_idx = np.arange(QB)[None, :]
    masks = np.stack(
        [(q_idx >= j * 128 + k_idx).astype(np.float32) for j in range(NJ)],
        axis=1)                                        # [128, NJ, QB]
    masks = np.ascontiguousarray(masks)

    in_maps = []
    for c in range(NCORES):
        wq_t = Wq[QR * c:QR * (c + 1)].T               # [HIDDEN, 512]
        wkr_c = Wkr[KRR * c:KRR * (c + 1)]             # [64, HIDDEN]
        wfk_c = Wupk[KRR * c:KRR * (c + 1)] @ Wdk      # [64, HIDDEN] fused
        wfv_c = Wupv[HEAD_DIM * c:HEAD_DIM * (c + 1)] @ Wdk  # [128, HIDDEN]
        kvrows = np.empty((128, HIDDEN), np.float32)
        kvrows[0:32] = wkr_c[0:32]     # krA: rope dims 0:32
        kvrows[32:64] = wfk_c[0:32]    # kcA: nope dims 32:64
        kvrows[64:96] = wkr_c[32:64]   # krB: rope dims 64:96
        kvrows[96:128] = wfk_c[32:64]  # kcB: nope dims 96:128
        wcomb = np.concatenate([wq_t, kvrows.T, wfv_c.T], axis=1)  # [HIDDEN, 768]
        wo_t = Wo[:, QR * c:QR * (c + 1)].T            # [512, HIDDEN]
        in_maps.append({
            "hidT": hidT,
            "wcomb": np.ascontiguousarray(wcomb).astype(bf16),
            "wo_t": np.ascontiguousarray(wo_t).astype(bf16),
            "ropes": ropes,
            "masks": masks,
            "ones": np.ones((128, 1), np.float32),
            "ident": np.eye(128, dtype=np.float32),
        })
    return in_maps


_NC_CACHE = {}


def _get_program(key=(B, S, 512, 512)):
    if key not in _NC_CACHE:
        _NC_CACHE[key] = build_program(*key)
    return _NC_CACHE[key]


def kernel(hidden_states, Wq, Wkr, Wdk, Wupk, Wupv, Wo):
    from concourse.bass_utils import run_bass_kernel_spmd

    in_maps = make_in_maps(np.asarray(hidden_states), np.asarray(Wq),
                           np.asarray(Wkr), np.asarray(Wdk), np.asarray(Wupk),
                           np.asarray(Wupv), np.asarray(Wo))
    nc = _get_program()
    res = run_bass_kernel_spmd(nc, in_maps, list(range(NCORES)))
    out = res.results[0]["out_part"].astype(np.float32)
    for i in range(1, NCORES):
        out = out + res.results[i]["out_part"].astype(np.float32)
    return out.reshape(B, S, HIDDEN).astype(np.float32)


# revision 6
# speedup vs baseline: 1.5284x; 1.2684x over previous
"""MLA (CustomLlamaMLAForInfer) Trainium2 Bass kernel, v2.

Sharding: tensor-parallel over heads across 8 NeuronCores. Core c owns
kv-head c and q-heads [4c, 4c+4). Every core sees the full token stream
(B*S = 4096 tokens); o_proj is computed against the core's 512
head-dims, producing a partial [4096, 4096] bf16 output that the host
sums across the 8 cores.

v2 changes vs baseline:
  - Host fuses Wupk/Wupv through Wdk (k_c = hid @ (Wupk_c Wdk).T etc.),
    removing the replicated 512-dim latent projection and its DRAM
    round trip entirely.
  - Single phase-1 pass over hidT: one 6-bank PSUM group per token
    block produces q (4 tiles), interleaved k_rope/k_nope (1 tile,
    weight columns pre-permuted so no cross-partition moves at evict),
    vT (1 tile, PE-transposed to [tok, d]).
  - bf16 operands on the PE except p/v (f32r), halving DMA traffic.
  - qT stays resident in SBUF (no DRAM round trip).
  - Attention: scores for 2 k-tiles accumulate into one 2-bank PSUM
    tile, one wide exp (N=1024) per group; softmax denominators via
    ones-matmul; reciprocal_approx_fast instead of iterative reciprocal.
  - o_proj interleaved per (b, qb) block right after its 4 heads
    finish, sharing PSUM banks with the scores pool; qb descending so
    the wo prefetch hides under the deepest attention block.
"""

import numpy as np

HIDDEN = 4096
N_HEADS = 32
KV_HEADS = 8
HEAD_DIM = 128
LOW_RANK = 64
TOP_K_ROPE = 32
ROPE_THETA = 10000.0
B, S = 2, 2048
NCORES = 8
HPC = N_HEADS // NCORES          # q heads per core = 4
QR = HPC * HEAD_DIM              # q rows per core = 512
CD = LOW_RANK * KV_HEADS         # latent dim = 512
KRR = 2 * TOP_K_ROPE             # rope rows per kv head = 64
WKV = 256                        # fused kv out rows: kr 64 + kc 64 + v 128
WC = QR + WKV                    # combined projection out rows = 768


def _rope_tables(seq_len):
    inv = 1.0 / (ROPE_THETA ** (np.arange(0, HEAD_DIM, 2, dtype=np.float32) / HEAD_DIM))
    pos = np.arange(seq_len, dtype=np.float32)
    fr = np.outer(pos, inv)
    emb = np.concatenate([fr, fr], axis=-1)          # [S, 128]
    return (np.cos(emb).T.astype(np.float32),        # [128, S]
            np.sin(emb).T.astype(np.float32))


def build_program(Bv=B, Sv=S, TB=512, QB=512, trace_sim=False):
    from concourse import bacc, tile, mybir
    import concourse.bass as bass

    f32 = mybir.dt.float32
    F32R = mybir.dt.float32r
    BF16 = mybir.dt.bfloat16
    MS = bass.MemorySpace
    EXP = mybir.ActivationFunctionType.Exp

    NT = Bv * Sv                 # total tokens = 4096
    HT = HIDDEN // 128           # hidden tiles = 32
    NTB = NT // TB               # proj token blocks = 8
    NQB = Sv // QB               # q blocks per batch = 4
    NJ = QB // 128               # diagonal mask variants = 4
    NKT_B = Sv // 128            # k tiles per batch = 16
    QT = QR // 128               # q-head tiles per core = 4

    nc = bacc.Bacc("TRN2", target_bir_lowering=False, debug=False,
                   num_devices=NCORES)

    def din(name, shape, dt=BF16):
        return nc.dram_tensor(name, shape, dt, kind="ExternalInput").ap()

    hidT = din("hidT", [HIDDEN, NT])
    wcomb = din("wcomb", [HIDDEN, WC])
    wo = din("wo_t", [QR, HIDDEN])
    ropes = din("ropes", [128, 4, NT])   # 0=qcos 1=qsin 2=kcos 3=ksin
    masks = din("masks", [128, NJ, QB], f32)
    onesd = din("ones", [128, 1], f32)
    identd = din("ident", [128, 128], f32)
    outp = nc.dram_tensor("out_part", [NT, HIDDEN], BF16, kind="ExternalOutput").ap()

    with tile.TileContext(nc, trace_sim=trace_sim) as tc:
        with tc.tile_pool(name="persist", bufs=1) as pers:
            kT = pers.tile([128, NT], BF16, tag="kT")
            qT = pers.tile([128, QT, NT], BF16, tag="qT")
            v_tok = pers.tile([128, NT // 128, HEAD_DIM], F32R, tag="vtok")

            # ---------------- phase 1: fused projections of hidden ----------
            with tc.tile_pool(name="p1c", bufs=1) as cp, \
                 tc.tile_pool(name="hid", bufs=3) as hp, \
                 tc.tile_pool(name="rps", bufs=2) as rpp, \
                 tc.tile_pool(name="st1", bufs=2) as st, \
                 tc.tile_pool(name="ps1", bufs=6, space=MS.PSUM) as pp, \
                 tc.tile_pool(name="psT", bufs=2, space=MS.PSUM) as pvt:
                ident_sb = cp.tile([128, 128], F32R, tag="id")
                nc.sync.dma_start(ident_sb[:], identd.bitcast(F32R))
                wc_sb = cp.tile([128, HT, WC], BF16, tag="wc")
                wc_r = wcomb.rearrange("(t p) w -> p t w", p=128)
                nc.sync.dma_start(wc_sb[:, 0:HT // 2, :], wc_r[:, 0:HT // 2, :])
                nc.sync.dma_start(wc_sb[:, HT // 2:HT, :], wc_r[:, HT // 2:HT, :])

                for blk in range(NTB):
                    c0, c1 = blk * TB, (blk + 1) * TB
                    rp = rpp.tile([128, 4, TB], BF16, tag="rp")
                    nc.sync.dma_start(rp[:], ropes[:, :, c0:c1])
                    hts = []
                    for half in range(2):
                        ht = hp.tile([128, HT // 2, TB], BF16, tag="hid")
                        nc.sync.dma_start(
                            ht[:],
                            hidT[half * 2048:(half + 1) * 2048, c0:c1]
                            .rearrange("(t p) w -> p t w", p=128))
                        hts.append(ht)
                    ps = [pp.tile([128, TB], f32, tag="ps1", name=f"ps{_m}")
                          for _m in range(6)]
                    for t in range(HT):
                        htt = hts[t // 16][:, t % 16, :]
                        for m in range(6):
                            nc.tensor.matmul(
                                ps[m][:],
                                wc_sb[:, t, m * 128:(m + 1) * 128],
                                htt,
                                start=(t == 0), stop=(t == HT - 1))
                    # ---- evict q tiles (rope via sign-folded tables) ----
                    for m in range(QT):
                        qraw = st.tile([128, TB], BF16, tag="qraw")
                        nc.scalar.copy(qraw[:], ps[m][:])
                        rot = st.tile([128, TB], BF16, tag="rot")
                        nc.sync.dma_start(rot[0:64, :], qraw[64:128, :])
                        nc.sync.dma_start(rot[64:128, :], qraw[0:64, :])
                        t1 = st.tile([128, TB], BF16, tag="t1")
                        nc.vector.tensor_mul(t1[:], qraw[:], rp[:, 0, :])
                        nc.vector.tensor_mul(rot[:], rot[:], rp[:, 1, :])
                        nc.vector.tensor_add(qT[:, m, c0:c1], t1[:], rot[:])
                    # ---- evict kv tile: rows 0:32 krA, 32:64 kcA,
                    #      64:96 krB, 96:128 kcB (host pre-permuted) ----
                    kk = st.tile([128, TB], BF16, tag="kk")
                    nc.scalar.copy(kk[:], ps[4][:])
                    rt = st.tile([128, TB], BF16, tag="rt")
                    nc.sync.dma_start(rt[0:32, :], kk[64:96, :])
                    nc.sync.dma_start(rt[64:96, :], kk[0:32, :])
                    t2 = st.tile([128, TB], BF16, tag="t2")
                    for r0 in (0, 64):
                        nc.vector.tensor_mul(
                            t2[r0:r0 + 32, :], kk[r0:r0 + 32, :],
                            rp[r0:r0 + 32, 2, :])
                        nc.vector.tensor_mul(
                            rt[r0:r0 + 32, :], rt[r0:r0 + 32, :],
                            rp[r0:r0 + 32, 3, :])
                        nc.vector.tensor_add(
                            kT[r0:r0 + 32, c0:c1], t2[r0:r0 + 32, :],
                            rt[r0:r0 + 32, :])
                    nc.scalar.copy(kT[32:64, c0:c1], kk[32:64, :])
                    nc.scalar.copy(kT[96:128, c0:c1], kk[96:128, :])
                    # ---- evict v tile: vT [d, tok] -> transpose to [tok, d] --
                    vst = st.tile([128, TB], F32R, tag="vst")
                    nc.scalar.copy(vst[:], ps[5][:])
                    for tt in range(TB // 128):
                        vtp = pvt.tile([128, 128], F32R, tag="vtp")
                        nc.tensor.transpose(
                            vtp[:], vst[:, tt * 128:(tt + 1) * 128], ident_sb[:])
                        nc.vector.tensor_copy(
                            v_tok[:, blk * (TB // 128) + tt, :], vtp[:])

            # ---------------- phases 3+4 interleaved ----------------
            with tc.tile_pool(name="wop", bufs=1) as wop, \
                 tc.tile_pool(name="attn", bufs=1) as ap_, \
                 tc.tile_pool(name="c3", bufs=1) as cst3, \
                 tc.tile_pool(name="pt", bufs=3) as ptp, \
                 tc.tile_pool(name="sm", bufs=2) as smp, \
                 tc.tile_pool(name="st4", bufs=2) as st4, \
                 tc.tile_pool(name="psS", bufs=2, space=MS.PSUM) as psS, \
                 tc.tile_pool(name="psO", bufs=2, space=MS.PSUM) as psO, \
                 tc.tile_pool(name="psU", bufs=2, space=MS.PSUM) as psU:
                masks_sb = cst3.tile([128, NJ, QB], F32R, tag="masks")
                nc.sync.dma_start(masks_sb[:], masks.bitcast(F32R))
                ones_sb = cst3.tile([128, 1], F32R, tag="ones")
                nc.sync.dma_start(ones_sb[:], onesd.bitcast(F32R))
                wo_sb = wop.tile([128, QT, HIDDEN], BF16, tag="wo")
                nc.sync.dma_start(wo_sb[:], wo.rearrange("(t p) w -> p t w", p=128))
                attn_sb = ap_.tile([128, QT, NT], BF16, tag="attn")

                def emit_attn(b, qb):
                    off = b * Sv
                    q0 = off + qb * QB
                    ngrp = NJ * (qb + 1) // 2   # groups of 2 k-tiles
                    for h in range(QT):
                        ops = psO.tile([128, QB], f32, tag="ops")
                        sps = psU.tile([1, QB], f32, tag="sps")
                        for g in range(ngrp):
                            scp = psS.tile([128, 2, QB], f32, tag="scp")
                            for j2 in range(2):
                                kt = 2 * g + j2
                                j = kt - NJ * qb        # diag idx if >= 0
                                tr = j * 128 if j > 0 else 0  # trim cols
                                nc.tensor.matmul(
                                    scp[:, j2, tr:QB],
                                    kT[:, off + kt * 128: off + (kt + 1) * 128],
                                    qT[:, h, q0 + tr:q0 + QB],
                                    start=True, stop=True)
                            ptile = ptp.tile([128, 2, QB], F32R, tag="pt")
                            nc.scalar.activation(ptile[:], scp[:], EXP)
                            dg = g - NJ * qb // 2
                            if dg >= 0:   # diagonal groups: causal mask
                                nc.vector.tensor_mul(
                                    ptile[:], ptile[:],
                                    masks_sb[:, 2 * dg:2 * dg + 2, :])
                            for j2 in range(2):
                                kt = 2 * g + j2
                                j = kt - NJ * qb
                                tr = j * 128 if j > 0 else 0
                                first = (g == 0 and j2 == 0)
                                last = (g == ngrp - 1 and j2 == 1)
                                nc.tensor.matmul(
                                    ops[:, tr:QB],
                                    v_tok[:, b * NKT_B + kt, :],
                                    ptile[:, j2, tr:QB],
                                    start=first, stop=last)
                                nc.tensor.matmul(
                                    sps[:, tr:QB],
                                    ones_sb[:],
                                    ptile[:, j2, tr:QB],
                                    start=first, stop=last)
                        rec = smp.tile([1, QB], f32, tag="rec")
                        nc.vector.reciprocal_approx_fast(rec[:], sps[:])
                        rb = smp.tile([128, QB], f32, tag="rb")
                        nc.gpsimd.partition_broadcast(rb[:], rec[:])
                        nc.vector.tensor_mul(
                            attn_sb[:, h, q0:q0 + QB], ops[:], rb[:])

                def emit_oproj(b, qb):
                    q0 = b * Sv + qb * QB
                    for Tt in range(QB // 128):
                        T0 = q0 + Tt * 128
                        orow = st4.tile([128, HIDDEN], BF16, tag="orow")
                        for npair in range(HIDDEN // 1024):
                            po = psS.tile([128, 2, QB], f32, tag="scp")
                            for half in range(2):
                                nn = 2 * npair + half
                                for h2 in range(QT):
                                    nc.tensor.matmul(
                                        po[:, half, :],
                                        attn_sb[:, h2, T0:T0 + 128],
                                        wo_sb[:, h2, nn * 512:(nn + 1) * 512],
                                        start=(h2 == 0), stop=(h2 == QT - 1))
                            # alternate eviction engine to balance load
                            if npair % 2 == 0:
                                nc.vector.tensor_copy(
                                    orow[:, npair * 1024:(npair + 1) * 1024],
                                    po[:])
                            else:
                                nc.scalar.copy(
                                    orow[:, npair * 1024:(npair + 1) * 1024],
                                    po[:])
                        nc.sync.dma_start(outp[T0:T0 + 128, :], orow[:])

                # o_proj pipelined one block behind attention so the PE never
                # waits on the softmax-normalize tail of the current block
                blocks = [(b, qb) for b in range(Bv)
                          for qb in range(NQB - 1, -1, -1)]
                prev = None
                for blk in blocks:
                    emit_attn(*blk)
                    if prev is not None:
                        emit_oproj(*prev)
                    prev = blk
                emit_oproj(*prev)

    nc.compile()
    return nc


def make_in_maps(hidden_states, Wq, Wkr, Wdk, Wupk, Wupv, Wo, Bv=B, Sv=S, QB=512):
    """Host-side sharding + layout prep. Returns per-core input dicts."""
    import ml_dtypes
    bf16 = ml_dtypes.bfloat16

    NT = Bv * Sv
    NJ = QB // 128
    scale = 1.0 / np.sqrt(np.float32(HEAD_DIM))

    hidden_states = np.asarray(hidden_states, dtype=np.float32)
    Wq = np.asarray(Wq, dtype=np.float32)
    Wkr = np.asarray(Wkr, dtype=np.float32)
    Wdk = np.asarray(Wdk, dtype=np.float32)
    Wupk = np.asarray(Wupk, dtype=np.float32)
    Wupv = np.asarray(Wupv, dtype=np.float32)
    Wo = np.asarray(Wo, dtype=np.float32)

    hidT = np.ascontiguousarray(
        hidden_states.reshape(NT, HIDDEN).T).astype(bf16)

    cos_t, sin_t = _rope_tables(Sv)                    # [128, S]
    cos_t = np.tile(cos_t, (1, Bv))                    # [128, NT]
    sin_t = np.tile(sin_t, (1, Bv))
    qcos = cos_t * scale
    qsin = np.concatenate([-sin_t[0:64], sin_t[64:128]], axis=0) * scale
    # k rope tables in the permuted kv-row layout:
    #   rows 0:32  = rotated rope dims 0:32   (krA):  cos[0:32],  -sin[0:32]
    #   rows 64:96 = rotated rope dims 64:96  (krB):  cos[64:96], +sin[64:96]
    kcos = np.zeros((128, NT), np.float32)
    ksin = np.zeros((128, NT), np.float32)
    kcos[0:32] = cos_t[0:32]
    kcos[64:96] = cos_t[64:96]
    ksin[0:32] = -sin_t[0:32]
    ksin[64:96] = sin_t[64:96]
    ropes = np.ascontiguousarray(
        np.stack([qcos, qsin, kcos, ksin], axis=1)).astype(bf16)  # [128,4,NT]

    k_idx = np.arange(128)[:, None]
    q_idx = np.arange(QB)[None, :]
    masks = np.stack(
        [(q_idx >= j * 128 + k_idx).astype(np.float32) for j in range(NJ)],
        axis=1)                                        # [128, NJ, QB]
    masks = np.ascontiguousarray(masks)

    in_maps = []
    for c in range(NCORES):
        wq_t = Wq[QR * c:QR * (c + 1)].T               # [HIDDEN, 512]
        wkr_c = Wkr[KRR * c:KRR * (c + 1)]             # [64, HIDDEN]
        wfk_c = Wupk[KRR * c:KRR * (c + 1)] @ Wdk      # [64, HIDDEN] fused
        wfv_c = Wupv[HEAD_DIM * c:HEAD_DIM * (c + 1)] @ Wdk  # [128, HIDDEN]
        kvrows = np.empty((128, HIDDEN), np.float32)
        kvrows[0:32] = wkr_c[0:32]     # krA: rope dims 0:32
        kvrows[32:64] = wfk_c[0:32]    # kcA: nope dims 32:64
        kvrows[64:96] = wkr_c[32:64]   # krB: rope dims 64:96
        kvrows[96:128] = wfk_c[32:64]  # kcB: nope dims 96:128
        wcomb = np.concatenate([wq_t, kvrows.T, wfv_c.T], axis=1)  # [HIDDEN, 768]
        wo_t = Wo[:, QR * c:QR * (c + 1)].T            # [512, HIDDEN]
        in_maps.append({
            "hidT": hidT,
            "wcomb": np.ascontiguousarray(wcomb).astype(bf16),
            "wo_t": np.ascontiguousarray(wo_t).astype(bf16),
            "ropes": ropes,
            "masks": masks,
            "ones": np.ones((128, 1), np.float32),
            "ident": np.eye(128, dtype=np.float32),
        })
    return in_maps


_NC_CACHE = {}


def _get_program(key=(B, S, 512, 512)):
    if key not in _NC_CACHE:
        _NC_CACHE[key] = build_program(*key)
    return _NC_CACHE[key]


def kernel(hidden_states, Wq, Wkr, Wdk, Wupk, Wupv, Wo):
    from concourse.bass_utils import run_bass_kernel_spmd

    in_maps = make_in_maps(np.asarray(hidden_states), np.asarray(Wq),
                           np.asarray(Wkr), np.asarray(Wdk), np.asarray(Wupk),
                           np.asarray(Wupv), np.asarray(Wo))
    nc = _get_program()
    res = run_bass_kernel_spmd(nc, in_maps, list(range(NCORES)))
    out = res.results[0]["out_part"].astype(np.float32)
    for i in range(1, NCORES):
        out = out + res.results[i]["out_part"].astype(np.float32)
    return out.reshape(B, S, HIDDEN).astype(np.float32)
